# revision 4
# baseline (speedup 1.0000x reference)
"""v8: multi-head hash kernel matched to measured trn2 engine semantics.

out[e,h] = (id0 ^ id1*q1[h] ^ id2*q2[h] ^ id3*q3[h]) & 0xFFFFF, masked.

Measured machine facts driving this design:
  - int32 bitwise ops exist ONLY on DVE; DVE int32 mult/ts-mult are fp32-routed
    (exact < 2^24 only) -> m2 (2^29) and m3 (wraps 2^32) need Pool's true
    integer multiplier.
  - Pool and DVE serialize EXACTLY (shared SBUF port pair, instruction-scoped
    lock) for every op shape tested -> total = Pool-time + DVE-time.
  - ACT never contends: m1 = id1*q1[h] (product < 2^22, fp32-exact) runs there.
So per iteration the serial resource carries: Pool m2+m3 (2 big bcast TTs,
~27.7us) + DVE 3 fused (&M, ^) STT folds (~26.4us).

Host-level dispatch: the mask is checked on host; all-ones (the harness case)
uses the fast build with no premasks and no mask DMA. A masked build handles
general masks (premask all 4 id planes on DVE).

Output is int32 h-major [P, H, KTOT]; host transposes to [B, S, H] and widens
to int64 (hashes < 2^20; pure format work).
"""
import sys

for _p in ("/opt/trn_rl_repo", "/root/.axon_site/_ro/trn_rl_repo"):
    if _p not in sys.path:
        sys.path.append(_p)

import numpy as np

B, S, O, H = 64, 8192, 4, 16
NCORES = 8
BPC = B // NCORES
P = 128
KTOT = BPC * S // P            # 512
TABLE = 1 << 20
MASK20 = TABLE - 1

_cache = {}


def _build(q1, q2, q3, iters=1, masked=False):
    import concourse.bass as bass
    from concourse import mybir

    A = mybir.AluOpType
    I32 = mybir.dt.int32
    U8 = mybir.dt.uint8
    KC = KTOT

    nc = bass.Bass()

    ids_d = nc.declare_dram_parameter("ids", [P, 4, KTOT], I32, isOutput=False)
    if masked:
        msk_d = nc.declare_dram_parameter("msk", [P, KTOT], U8, isOutput=False)
    cst_d = nc.declare_dram_parameter("cst", [P, 2 * H], I32, isOutput=False)
    out_d = nc.declare_dram_parameter("out", [P, H, KTOT], I32, isOutput=True)

    raw = [nc.alloc_sbuf_tensor(f"raw{x}", [P, 4, KC], I32) for x in range(2)]
    if masked:
        mk8 = [nc.alloc_sbuf_tensor(f"mk8{x}", [P, KC], U8) for x in range(2)]
        idm = [[nc.alloc_sbuf_tensor(f"id{i}m{x}", [P, KC], I32) for i in range(4)] for x in range(2)]
    cst = nc.alloc_sbuf_tensor("cst_t", [P, 2 * H], I32)
    mA = nc.alloc_sbuf_tensor("mA", [P, 1], I32)
    m1b = nc.alloc_sbuf_tensor("m1b", [P, H, KC], I32)
    m2b = nc.alloc_sbuf_tensor("m2b", [P, H, KC], I32)
    m3b = nc.alloc_sbuf_tensor("m3b", [P, H, KC], I32)
    ot = [nc.alloc_sbuf_tensor(f"ot{x}", [P, H, KC], I32) for x in range(2)]

    s_in = nc.alloc_semaphore("s_in")    # dma-in completions (16/dma)
    s_m23 = nc.alloc_semaphore("s_m23")  # Pool: +1 after m2(r), +1 after m3(r)
    s_m1 = nc.alloc_semaphore("s_m1")    # ACT m1(r) done
    s_pm = nc.alloc_semaphore("s_pm")    # masked only: +1 after premask block(r)
    s_ff = nc.alloc_semaphore("s_ff")    # +1 after fold pass2(r) (m1b consumed)
    s_f = nc.alloc_semaphore("s_f")      # +1 after fold pass3(r)
    s_out = nc.alloc_semaphore("s_out")  # dma-out completions (16/dma)

    cst2_b = cst[:, 0:H].rearrange("p (h x) -> p h x", x=1).broadcast_to([P, H, KC])
    cst3_b = cst[:, H:2 * H].rearrange("p (h x) -> p h x", x=1).broadcast_to([P, H, KC])

    def bcast(t):
        return t.rearrange("p (x k) -> p x k", x=1).broadcast_to([P, H, KC])

    DMAS = 2 if masked else 1

    def plane(x, i):
        """id plane i (premasked when masked)."""
        return idm[x][i][:] if masked else raw[x][:, i, :]

    with nc.Block() as block:
        @block.sync
        def _(sync: bass.BassEngine):
            sync.dma_start(out=cst[:], in_=cst_d[:]).then_inc(s_in, 16)
            for k in range(min(2, iters)):
                sync.dma_start(out=raw[k % 2][:], in_=ids_d[:]).then_inc(s_in, 16)
                if masked:
                    sync.dma_start(out=mk8[k % 2][:], in_=msk_d[:]).then_inc(s_in, 16)
            for r in range(iters):
                k = r + 2
                if k < iters:
                    # raw[k%2] free once iter k-2 is fully done
                    sync.wait_ge(s_f, k - 1)
                    sync.dma_start(out=raw[k % 2][:], in_=ids_d[:]).then_inc(s_in, 16)
                    if masked:
                        sync.dma_start(out=mk8[k % 2][:], in_=msk_d[:]).then_inc(s_in, 16)
                sync.wait_ge(s_f, r + 1)
                sync.dma_start(out=out_d[:], in_=ot[r % 2][:]).then_inc(s_out, 16)
            sync.wait_ge(s_out, 16 * iters)

        @block.gpsimd
        def _(gp: bass.BassEngine):
            for r in range(iters):
                x = r % 2
                if masked:
                    gp.wait_ge(s_pm, r + 1)
                else:
                    gp.wait_ge(s_in, 16 + 16 * DMAS * (r + 1))
                if r >= 1:
                    gp.wait_ge(s_f, r)  # m2b/m3b consumed by pass3(r-1)
                gp.tensor_tensor(m2b[:], bcast(plane(x, 2)), cst2_b, A.mult).then_inc(s_m23, 1)
                gp.tensor_tensor(m3b[:], bcast(plane(x, 3)), cst3_b, A.mult).then_inc(s_m23, 1)

        @block.vector
        def _(v: bass.BassEngine):
            v.memset(mA[:], MASK20)
            for r in range(iters):
                x = r % 2
                if masked:
                    v.wait_ge(s_in, 16 + 16 * DMAS * (r + 1))
                    if r >= 2:
                        v.wait_ge(s_m1, r - 1)  # ACT(r-2) done with idm[x][1]
                    v.tensor_tensor(idm[x][1][:], raw[x][:, 1, :], mk8[x][:], A.mult)
                    v.tensor_tensor(idm[x][2][:], raw[x][:, 2, :], mk8[x][:], A.mult)
                    v.tensor_tensor(idm[x][3][:], raw[x][:, 3, :], mk8[x][:], A.mult)
                    v.tensor_tensor(idm[x][0][:], raw[x][:, 0, :], mk8[x][:], A.mult).then_inc(s_pm, 1)
                v.wait_ge(s_m23, 2 * r + 1)
                v.scalar_tensor_tensor(m2b[:], m2b[:], mA[:], bcast(plane(x, 0)),
                                       A.bitwise_and, A.bitwise_xor)
                v.wait_ge(s_m1, r + 1)
                v.scalar_tensor_tensor(m2b[:], m1b[:], mA[:], m2b[:],
                                       A.bitwise_and, A.bitwise_xor).then_inc(s_ff, 1)
                v.wait_ge(s_m23, 2 * r + 2)
                if r >= 2:
                    v.wait_ge(s_out, 16 * (r - 1))  # ot[x] drained
                v.scalar_tensor_tensor(ot[x][:], m3b[:], mA[:], m2b[:],
                                       A.bitwise_and, A.bitwise_xor).then_inc(s_f, 1)

        @block.scalar
        def _(sc: bass.BassEngine):
            for r in range(iters):
                x = r % 2
                if masked:
                    sc.wait_ge(s_pm, r + 1)
                else:
                    sc.wait_ge(s_in, 16 + 16 * DMAS * (r + 1))
                if r >= 1:
                    sc.wait_ge(s_ff, r)  # pass2(r-1) consumed m1b
                for h in range(H):
                    ins = sc.mul(m1b[:, h, :], plane(x, 1), float(q1[h]))
                    if h == H - 1:
                        ins.then_inc(s_m1, 1)

    return nc


def _prep_inputs(ids, msk, q1, q2, q3, masked=False):
    ids32 = ids.view(np.int32).reshape(B, S, 2 * O)[:, :, 0::2]   # low words

    cstv = np.empty((P, 2 * H), np.int32)
    cstv[:, :H] = np.asarray(q2, np.int64).astype(np.int32)[None, :]
    cstv[:, H:] = np.asarray(q3, np.uint32).view(np.int32)[None, :]

    if masked:
        msk8 = np.ascontiguousarray(msk).astype(np.uint8, copy=False)

    in_maps = []
    for c in range(NCORES):
        core_ids = ids32[c * BPC:(c + 1) * BPC].reshape(P, KTOT, 4)
        core_ids = np.ascontiguousarray(core_ids.transpose(0, 2, 1))  # [P, 4, KTOT]
        m = {"ids": core_ids, "cst": cstv}
        if masked:
            m["msk"] = np.ascontiguousarray(msk8[c * BPC:(c + 1) * BPC]).reshape(P, KTOT)
        in_maps.append(m)
    return in_maps


def kernel(ngram_ids, ngram_mask, prime_powers, table_size):
    from concourse.bass_utils import run_bass_kernel_spmd

    ids = np.asarray(ngram_ids)
    msk = np.asarray(ngram_mask)
    pw = np.asarray(prime_powers)
    assert int(table_size) == TABLE
    assert ids.shape == (B, S, O) and ids.dtype == np.int64
    assert pw.shape[1] >= 4 and np.all(pw[:, 0] == 1)

    q1 = [int(x) for x in pw[:H, 1]]
    q2 = [int(x) for x in pw[:H, 2]]
    q3 = [int(x & 0xFFFFFFFF) for x in pw[:H, 3]]

    masked = not bool(np.all(msk))
    key = (tuple(q1), tuple(q2), tuple(q3), masked)
    if key not in _cache:
        _cache[key] = _build(q1, q2, q3, masked=masked)
    nc = _cache[key]

    in_maps = _prep_inputs(ids, msk, q1, q2, q3, masked=masked)
    res = run_bass_kernel_spmd(nc, in_maps, list(range(NCORES)))

    out = np.empty((B, S, H), np.int64)
    for c in range(NCORES):
        o32 = res.results[c]["out"]                       # [P, H, KTOT] int32
        arr = o32.reshape(BPC, P // BPC, H, KTOT).transpose(0, 1, 3, 2)
        out[c * BPC:(c + 1) * BPC] = arr.reshape(BPC, S, H).astype(np.int64)
    return out


if __name__ == "__main__":
    rng = np.random.default_rng(0)
    ids = rng.integers(0, 32000, size=(B, S, O)).astype(np.int64)
    primes = np.array([31, 37, 41, 43, 47, 53, 59, 61, 67, 71, 73, 79, 83, 89, 97, 101], np.int64)
    pw = primes[:, None] ** np.arange(8, dtype=np.int64)[None, :]

    def expected(ids, msk):
        w = ids[:, :, :, None].astype(np.int64) * pw.T[:4][None, None, :, :]
        exp = w[..., 0, :]
        for i in range(1, 4):
            exp = exp ^ w[..., i, :]
        return (exp % TABLE) * msk[..., None]

    for name, msk in [("ones", np.ones((B, S), dtype=bool)),
                      ("holes", None)]:
        if msk is None:
            msk = np.ones((B, S), dtype=bool)
            msk[3, 100:200] = False
        got = kernel(ids, msk, pw, TABLE)
        exp = expected(ids, msk)
        ok = np.array_equal(got, exp)
        print(f"match[{name}]:", ok)
        if not ok:
            bad = got != exp
            idx = np.argwhere(bad)
            print("nbad:", len(idx))
            for b_, s_, h_ in idx[:5]:
                print(b_, s_, h_, got[b_, s_, h_], exp[b_, s_, h_])


# revision 5
# speedup vs baseline: 2.4868x; 2.4868x over previous
"""v9: multi-head hash kernel matched to measured trn2 engine semantics.

out[e,h] = (id0 ^ id1*q1[h] ^ id2*q2[h] ^ id3*q3[h]) & 0xFFFFF, masked.

Measured machine facts driving this design:
  - int32 bitwise ops exist ONLY on DVE; DVE int32 mult is fp32-routed
    (exact < 2^24 only) -> m2 (2^29) and m3 (wraps 2^32) need Pool's true
    integer multiplier.
  - Pool and DVE serialize EXACTLY (shared SBUF port pair): wall time =
    Pool-busy + DVE-busy. The scheduling goal is zero bubbles on that
    combined resource, not overlap.
  - ACT never contends: m1 = id1*q1[h] (product < 2^22, fp32-exact).
Fold chain ordered so m3 is consumed LAST (Pool can compute m3(r) and
m2(r+1) while DVE folds iteration r):
  p1: m2b = (m2b & M) ^ id0_bcast      (in-place)
  p2: m2b = (m1b & M) ^ m2b            (in-place)
  p3: ot  = (m3b & M) ^ m2b
Host dispatch: all-ones mask (the harness case) -> fast build without
premasks/mask-DMA; general masks -> masked build (premask id planes).
Output int32 h-major [P, H, KTOT]; host transposes to [B, S, H] + widens
to int64 (hashes < 2^20; pure format work).
"""
import sys

for _p in ("/opt/trn_rl_repo", "/root/.axon_site/_ro/trn_rl_repo"):
    if _p not in sys.path:
        sys.path.append(_p)

import numpy as np

B, S, O, H = 64, 8192, 4, 16
NCORES = 8
BPC = B // NCORES
P = 128
KTOT = BPC * S // P            # 512
KH = KTOT // 2                 # half-tile slice for finer interleave
TABLE = 1 << 20
MASK20 = TABLE - 1

_cache = {}


def _build(q1, q2, q3, iters=1, masked=False):
    import concourse.bass as bass
    from concourse import mybir

    A = mybir.AluOpType
    I32 = mybir.dt.int32
    U8 = mybir.dt.uint8
    KC = KTOT

    nc = bass.Bass()

    ids_d = nc.declare_dram_parameter("ids", [P, 4, KTOT], I32, isOutput=False)
    if masked:
        msk_d = nc.declare_dram_parameter("msk", [P, KTOT], U8, isOutput=False)
    cst_d = nc.declare_dram_parameter("cst", [P, 2 * H], I32, isOutput=False)
    out_d = nc.declare_dram_parameter("out", [P, H, KTOT], I32, isOutput=True)

    raw = [nc.alloc_sbuf_tensor(f"raw{x}", [P, 4, KC], I32) for x in range(2)]
    if masked:
        mk8 = [nc.alloc_sbuf_tensor(f"mk8{x}", [P, KC], U8) for x in range(2)]
        idm = [[nc.alloc_sbuf_tensor(f"id{i}m{x}", [P, KC], I32) for i in range(4)] for x in range(2)]
    cst = nc.alloc_sbuf_tensor("cst_t", [P, 2 * H], I32)
    mA = nc.alloc_sbuf_tensor("mA", [P, 1], I32)
    m1b = nc.alloc_sbuf_tensor("m1b", [P, H, KC], I32)
    m2b = [nc.alloc_sbuf_tensor(f"m2b{x}", [P, H, KC], I32) for x in range(2)]
    m3b = nc.alloc_sbuf_tensor("m3b", [P, H, KC], I32)
    ot = nc.alloc_sbuf_tensor("ot", [P, H, KC], I32)

    s_in = nc.alloc_semaphore("s_in")    # dma-in completions (16/dma)
    s_m2 = nc.alloc_semaphore("s_m2")    # +1 after Pool m2(r)
    s_m3 = nc.alloc_semaphore("s_m3")    # +1 after Pool m3(r)
    s_m1 = nc.alloc_semaphore("s_m1")    # +1 after ACT m1(r)
    s_pm = nc.alloc_semaphore("s_pm")    # masked only: +1 after premask block(r)
    s_ff = nc.alloc_semaphore("s_ff")    # +1 after p2(r)
    s_f = nc.alloc_semaphore("s_f")      # +1 after p3(r)
    s_out = nc.alloc_semaphore("s_out")  # dma-out completions (16/dma)

    cst2_b = cst[:, 0:H].rearrange("p (h x) -> p h x", x=1).broadcast_to([P, H, KC])
    cst3_b = cst[:, H:2 * H].rearrange("p (h x) -> p h x", x=1).broadcast_to([P, H, KC])

    DMAS = 2 if masked else 1

    def plane(x, i):
        return idm[x][i] if masked else raw[x][:, i, :]

    def bc(t, lo, hi):
        return t[:, lo:hi].rearrange("p (x k) -> p x k", x=1).broadcast_to([P, H, hi - lo])

    with nc.Block() as block:
        @block.sync
        def _(sync: bass.BassEngine):
            sync.dma_start(out=cst[:], in_=cst_d[:]).then_inc(s_in, 16)
            for k in range(min(2, iters)):
                sync.dma_start(out=raw[k % 2][:], in_=ids_d[:]).then_inc(s_in, 16)
                if masked:
                    sync.dma_start(out=mk8[k % 2][:], in_=msk_d[:]).then_inc(s_in, 16)
            for r in range(iters):
                k = r + 2
                if k < iters:
                    sync.wait_ge(s_ff, r + 1)  # p2(r) done => p1(r) consumed id0(r)
                    sync.dma_start(out=raw[k % 2][:], in_=ids_d[:]).then_inc(s_in, 16)
                    if masked:
                        sync.dma_start(out=mk8[k % 2][:], in_=msk_d[:]).then_inc(s_in, 16)
                sync.wait_ge(s_f, r + 1)
                sync.dma_start(out=out_d[:], in_=ot[:]).then_inc(s_out, 16)
            sync.wait_ge(s_out, 16 * iters)

        @block.gpsimd
        def _(gp: bass.BassEngine):
            def m2_op(r):
                x = r % 2
                if masked:
                    gp.wait_ge(s_pm, r + 1)
                else:
                    gp.wait_ge(s_in, 16 + 16 * DMAS * (r + 1))
                if r >= 2:
                    gp.wait_ge(s_f, r - 1)  # p3(r-2) consumed m2b[x]
                p = plane(x, 2)
                for lo in (0, KH):
                    ins = gp.tensor_tensor(m2b[x][:, :, lo:lo + KH], bc(p, lo, lo + KH),
                                           cst2_b[:, :, lo:lo + KH], A.mult)
                ins.then_inc(s_m2, 1)

            def m3_op(r):
                x = r % 2
                if masked:
                    gp.wait_ge(s_pm, r + 1)
                else:
                    gp.wait_ge(s_in, 16 + 16 * DMAS * (r + 1))
                if r >= 1:
                    gp.wait_ge(s_f, r)  # p3(r-1) consumed m3b
                p = plane(x, 3)
                for lo in (0, KH):
                    ins = gp.tensor_tensor(m3b[:, :, lo:lo + KH], bc(p, lo, lo + KH),
                                           cst3_b[:, :, lo:lo + KH], A.mult)
                ins.then_inc(s_m3, 1)

            m2_op(0)
            for r in range(iters):
                m3_op(r)
                if r + 1 < iters:
                    m2_op(r + 1)

        @block.vector
        def _(v: bass.BassEngine):
            v.memset(mA[:], MASK20)
            for r in range(iters):
                x = r % 2
                if masked:
                    v.wait_ge(s_in, 16 + 16 * DMAS * (r + 1))
                    if r >= 2:
                        v.wait_ge(s_m1, r - 1)
                    v.tensor_tensor(idm[x][1][:], raw[x][:, 1, :], mk8[x][:], A.mult)
                    v.tensor_tensor(idm[x][2][:], raw[x][:, 2, :], mk8[x][:], A.mult)
                    v.tensor_tensor(idm[x][3][:], raw[x][:, 3, :], mk8[x][:], A.mult)
                    v.tensor_tensor(idm[x][0][:], raw[x][:, 0, :], mk8[x][:], A.mult).then_inc(s_pm, 1)
                v.wait_ge(s_m2, r + 1)
                p0 = plane(x, 0)
                for lo in (0, KH):
                    v.scalar_tensor_tensor(m2b[x][:, :, lo:lo + KH], m2b[x][:, :, lo:lo + KH],
                                           mA[:], bc(p0, lo, lo + KH),
                                           A.bitwise_and, A.bitwise_xor)
                v.wait_ge(s_m1, r + 1)
                for lo in (0, KH):
                    ins = v.scalar_tensor_tensor(m2b[x][:, :, lo:lo + KH], m1b[:, :, lo:lo + KH],
                                                 mA[:], m2b[x][:, :, lo:lo + KH],
                                                 A.bitwise_and, A.bitwise_xor)
                ins.then_inc(s_ff, 1)
                v.wait_ge(s_m3, r + 1)
                if r >= 1:
                    v.wait_ge(s_out, 16 * r)  # ot drained by out-dma(r-1)
                for lo in (0, KH):
                    ins = v.scalar_tensor_tensor(ot[:, :, lo:lo + KH], m3b[:, :, lo:lo + KH],
                                                 mA[:], m2b[x][:, :, lo:lo + KH],
                                                 A.bitwise_and, A.bitwise_xor)
                ins.then_inc(s_f, 1)

        @block.scalar
        def _(sc: bass.BassEngine):
            for r in range(iters):
                x = r % 2
                if masked:
                    sc.wait_ge(s_pm, r + 1)
                else:
                    sc.wait_ge(s_in, 16 + 16 * DMAS * (r + 1))
                if r >= 1:
                    sc.wait_ge(s_ff, r)  # p2(r-1) consumed m1b
                for h in range(H):
                    ins = sc.mul(m1b[:, h, :], plane(x, 1), float(q1[h]))
                    if h == H - 1:
                        ins.then_inc(s_m1, 1)

    return nc


def _prep_inputs(ids, msk, q1, q2, q3, masked=False):
    ids32 = ids.view(np.int32).reshape(B, S, 2 * O)[:, :, 0::2]   # low words

    cstv = np.empty((P, 2 * H), np.int32)
    cstv[:, :H] = np.asarray(q2, np.int64).astype(np.int32)[None, :]
    cstv[:, H:] = np.asarray(q3, np.uint32).view(np.int32)[None, :]

    if masked:
        msk8 = np.ascontiguousarray(msk).astype(np.uint8, copy=False)

    in_maps = []
    for c in range(NCORES):
        core_ids = ids32[c * BPC:(c + 1) * BPC].reshape(P, KTOT, 4)
        core_ids = np.ascontiguousarray(core_ids.transpose(0, 2, 1))  # [P, 4, KTOT]
        m = {"ids": core_ids, "cst": cstv}
        if masked:
            m["msk"] = np.ascontiguousarray(msk8[c * BPC:(c + 1) * BPC]).reshape(P, KTOT)
        in_maps.append(m)
    return in_maps


def kernel(ngram_ids, ngram_mask, prime_powers, table_size):
    from concourse.bass_utils import run_bass_kernel_spmd

    ids = np.asarray(ngram_ids)
    msk = np.asarray(ngram_mask)
    pw = np.asarray(prime_powers)
    assert int(table_size) == TABLE
    assert ids.shape == (B, S, O) and ids.dtype == np.int64
    assert pw.shape[1] >= 4 and np.all(pw[:, 0] == 1)

    q1 = [int(x) for x in pw[:H, 1]]
    q2 = [int(x) for x in pw[:H, 2]]
    q3 = [int(x & 0xFFFFFFFF) for x in pw[:H, 3]]

    masked = not bool(np.all(msk))
    key = (tuple(q1), tuple(q2), tuple(q3), masked)
    if key not in _cache:
        _cache[key] = _build(q1, q2, q3, masked=masked)
    nc = _cache[key]

    in_maps = _prep_inputs(ids, msk, q1, q2, q3, masked=masked)
    res = run_bass_kernel_spmd(nc, in_maps, list(range(NCORES)))

    out = np.empty((B, S, H), np.int64)
    for c in range(NCORES):
        o32 = res.results[c]["out"]                       # [P, H, KTOT] int32
        arr = o32.reshape(BPC, P // BPC, H, KTOT).transpose(0, 1, 3, 2)
        out[c * BPC:(c + 1) * BPC] = arr.reshape(BPC, S, H).astype(np.int64)
    return out


if __name__ == "__main__":
    rng = np.random.default_rng(0)
    ids = rng.integers(0, 32000, size=(B, S, O)).astype(np.int64)
    primes = np.array([31, 37, 41, 43, 47, 53, 59, 61, 67, 71, 73, 79, 83, 89, 97, 101], np.int64)
    pw = primes[:, None] ** np.arange(8, dtype=np.int64)[None, :]

    def expected(ids, msk):
        w = ids[:, :, :, None].astype(np.int64) * pw.T[:4][None, None, :, :]
        exp = w[..., 0, :]
        for i in range(1, 4):
            exp = exp ^ w[..., i, :]
        return (exp % TABLE) * msk[..., None]

    for name, msk in [("ones", np.ones((B, S), dtype=bool)),
                      ("holes", None)]:
        if msk is None:
            msk = np.ones((B, S), dtype=bool)
            msk[3, 100:200] = False
        got = kernel(ids, msk, pw, TABLE)
        exp = expected(ids, msk)
        ok = np.array_equal(got, exp)
        print(f"match[{name}]:", ok)
        if not ok:
            bad = got != exp
            idx = np.argwhere(bad)
            print("nbad:", len(idx))
            for b_, s_, h_ in idx[:5]:
                print(b_, s_, h_, got[b_, s_, h_], exp[b_, s_, h_])
